# revision 13
# baseline (speedup 1.0000x reference)
"""Cross-covariance attention (XCA) kernel for Trainium2, 8 NeuronCores.

Problem (per batch element b, one per core — data-parallel over B=8):
    qkv = x @ Wqkv;  q,k,v heads of dim 64;  q,k L2-normalized over the
    TOKEN axis;  attn_h = softmax((k_h^T q_h) * temp_h) (64x64, head-local);
    y = concat_h(v_h @ attn_h) @ Wout + bout.

Algebraic reduction: the attention matrix only depends on the token
covariance C = x^T x (768x768):
    k_h^T q_h = Wk_h^T C Wq_h,   ||q_col_j||^2 = diag(Wq^T C Wq)_j
and the output collapses to y = x @ W3 + bout with
    W3 = Wv @ blockdiag(A_h) @ Wout.

v4 design (trace-driven, vs the 270us baseline):
- SWDGE (gpsimd) dma_starts serialize at ~8-10us each regardless of
  descriptor size, so the x stream moves to the two HWDGE queues
  (sync/scalar) as RAW fp32 loads with 12KB-contiguous descriptors
  ("(p tl) c": consecutive token rows per partition; the permutation is
  free for C and inverted by the y-tile writes).  fp16 (for the XBAR
  x^T transpose) and fp8 (for C) copies are produced by DVE/ACT,
  alternating engines per group.  gpsimd carries only the weight
  loads, weights-first, so Wqk lands by ~15us instead of ~115us.
- C = x^T x runs in fp8e4 DoubleRow (256-token contraction per pass,
  upper block-triangle + PE-transpose mirror), ~20us of PE instead of
  46us, which makes phase A cast/DMA-paced at ~35us.
- Mqk = (C/32) @ [Wq|Wk] in fp8 DoubleRow; the 1/32 scale cancels in
  the normalized logits.  nk norm reductions are interleaved into the
  Mqk stream (Mk is consumed from PSUM, never stored).
- Batched softmax across all 6 head pairs: compact [P,12] norm
  diagonals via tensor_tensor_reduce against the identity, DVE
  reciprocals only on [P,12], q-side 1/nq replicated via an
  identity-matmul broadcast, one [P,768] exp (ACT tables pre-warmed),
  masked block-diagonal A with fused mask+rowsum, 6 PE transposes,
  W2 = A^T-blocks @ Wout, W3 = Wv @ W2, y = x @ W3 + bout.
- PE warmup burst at t=0 so the HAM clock gate reaches 8/8 early; all
  PE idle gaps are kept under the ~3.4us re-throttle window.
"""

import numpy as np

import concourse.bacc as bacc
import concourse.bass as bass
import concourse.mybir as mybir
import concourse.tile as tile
from concourse.bass_utils import run_bass_kernel_spmd
from concourse.masks import make_identity

F32 = mybir.dt.float32
FP16 = mybir.dt.float16
FP8 = mybir.dt.float8e4

B, N, D = 8, 4096, 768
H, DH = 12, 64
P = 128
KT = D // P  # 6 feature tiles
TT = N // P  # 32 token tiles
GRP = 4  # token tiles per load group
GT = TT // GRP  # 8 groups
HP = H // 2  # 6 head pairs (2 heads packed into 128 partitions)
EPS = 1e-12
CS = 1.0 / 32.0  # C -> fp8 scale (cancels in normalized logits)
DR = mybir.MatmulPerfMode.DoubleRow
Copy = mybir.ActivationFunctionType.Copy

import os

C_FP8 = os.environ.get("BASS_C_FP8", "1") == "1"
DEBUG_DUMP = os.environ.get("BASS_DEBUG", "0") == "1"
USE_TTR = os.environ.get("BASS_TTR", "1") == "1"
USE_WARMACT = os.environ.get("BASS_WARMACT", "1") == "1"
USE_ACTC8 = os.environ.get("BASS_ACTC8", "1") == "1"


def _diag_extract(nc, dscr_hp, src_ap, ident32, accum):
    if USE_TTR:
        nc.vector.tensor_tensor_reduce(
            dscr_hp, src_ap, ident32, 1.0, 0.0,
            op0=mybir.AluOpType.mult, op1=mybir.AluOpType.add,
            accum_out=accum,
        )
    else:
        nc.vector.tensor_mul(dscr_hp, src_ap, ident32)
        nc.vector.tensor_reduce(
            accum, dscr_hp, axis=mybir.AxisListType.X, op=mybir.AluOpType.add
        )


def build_nc():
    nc = bacc.Bacc("TRN2", target_bir_lowering=False, debug=False)

    x_d = nc.dram_tensor("x", (N, D), F32, kind="ExternalInput")
    wqkv_d = nc.dram_tensor("wqkv", (D, 3 * D), F32, kind="ExternalInput")
    temp_d = nc.dram_tensor("temp", (H,), F32, kind="ExternalInput")
    wout_d = nc.dram_tensor("wout", (D, D), F32, kind="ExternalInput")
    bout_d = nc.dram_tensor("bout", (D,), F32, kind="ExternalInput")
    y_d = nc.dram_tensor("y", (N, D), F32, kind="ExternalOutput")
    dbg = {}
    if DEBUG_DUMP:
        dbg["c8"] = nc.dram_tensor("dbg_c8", (D, D), F32, kind="ExternalOutput")
        dbg["mq"] = nc.dram_tensor("dbg_mq", (D, D), F32, kind="ExternalOutput")
        dbg["a"] = nc.dram_tensor("dbg_a", (P, HP * P), F32, kind="ExternalOutput")
        dbg["w3"] = nc.dram_tensor("dbg_w3", (D, D), F32, kind="ExternalOutput")
        dbg["nrm"] = nc.dram_tensor("dbg_nrm", (P, 4 * HP), F32, kind="ExternalOutput")
        dbg["at"] = nc.dram_tensor("dbg_at", (P, HP * P), F32, kind="ExternalOutput")
        dbg["w2"] = nc.dram_tensor("dbg_w2", (D, D), F32, kind="ExternalOutput")
        dbg["wvt"] = nc.dram_tensor("dbg_wvt", (D, D), F32, kind="ExternalOutput")

    with tile.TileContext(nc) as tc:
        _emit(tc, nc, x_d, wqkv_d, temp_d, wout_d, bout_d, y_d, dbg)
    nc.compile()
    return nc


def _emit(tc, nc, x_d, wqkv_d, temp_d, wout_d, bout_d, y_d, dbg={}):
    from contextlib import ExitStack

    ctx = ExitStack()
    with ctx:
        # ---------------- persistent pools ----------------
        persist = ctx.enter_context(tc.tile_pool(name="persist", bufs=1))
        small = ctx.enter_context(tc.tile_pool(name="small", bufs=1))

        xtt = persist.tile([P, TT, KT, P], FP16)  # x^T, tile-major
        wqk_sb = persist.tile([P, KT, 2 * D], FP16)  # [Wq | Wk]
        wqk8 = persist.tile([P, KT, 2 * D], FP8)  # [Wq | Wk] fp8
        c_sb = persist.tile([P, KT, D], FP16)  # C upper blocks (fp16)
        c8_sb = persist.tile([P, KT, D], FP8)  # C/32 full (fp8)
        mq_sb = persist.tile([P, KT, D], FP16)  # Mq/32 = (C/32) @ Wq
        wvt_sb = persist.tile([P, KT, D], FP16)  # Wv^T
        wout_sb = persist.tile([P, KT, D], FP16)  # Wout (natural)
        w2_sb = persist.tile([P, KT, D], FP16)  # blockdiag(A) @ Wout
        w3_sb = persist.tile([P, KT, D], FP16)  # W3 = Wv @ W2

        ident32 = small.tile([P, P], F32)
        make_identity(nc, ident32)
        ident16 = small.tile([P, P], FP16)
        nc.vector.tensor_copy(ident16, ident32)
        ones16 = small.tile([P, P], FP16)
        nc.vector.memset(ones16, 1.0)
        blockmask = small.tile([P, P], FP16)  # blockdiag(1_64, 1_64)
        nc.vector.memset(blockmask, 0.0)
        nc.vector.memset(blockmask[0:64, 0:64], 1.0)
        nc.vector.memset(blockmask[64:128, 64:128], 1.0)
        temp_sb = small.tile([P, H], F32)
        nc.gpsimd.dma_start(temp_sb, temp_d[None, :].to_broadcast((P, H)))
        bout_sb = small.tile([P, D], F32)
        nc.gpsimd.dma_start(bout_sb, bout_d[None, :].to_broadcast((P, D)))
        eps2 = small.tile([P, 1], F32)
        nc.vector.memset(eps2, EPS * EPS)
        dwarm = small.tile([P, 1], F32)
        nc.vector.memset(dwarm, 1.0)
        # tdiag[p, hp] = temp[2*hp + p//64] (partition-indexed temperature)
        tdiag = small.tile([P, HP], F32)
        for hp in range(HP):
            nc.vector.tensor_copy(
                tdiag[0:64, hp : hp + 1], temp_sb[0:64, 2 * hp : 2 * hp + 1]
            )
            nc.vector.tensor_copy(
                tdiag[64:128, hp : hp + 1],
                temp_sb[64:128, 2 * hp + 1 : 2 * hp + 2],
            )

        # weights on the gpsimd queue, weights-first (x goes via HWDGE)
        nc.gpsimd.dma_start(
            wqk_sb, wqkv_d[:, 0 : 2 * D].rearrange("(ko p) c -> p ko c", p=P)
        )

        # PE warmup: burn the HAM cold window on junk matmuls while the
        # first x group is still in flight.
        with tc.tile_pool(name="psWarm", bufs=1, space="PSUM") as psWarm:
            wps = psWarm.tile([P, P], F32)
            for _ in range(30):
                nc.tensor.matmul(wps, ones16, ones16, start=True, stop=True)

        with tc.tile_pool(name="wvpool", bufs=1) as wvpool:
            wv_sb = wvpool.tile([P, KT, D], FP16)
            nc.gpsimd.dma_start(
                wv_sb,
                wqkv_d[:, 2 * D : 3 * D].rearrange("(ko p) c -> p ko c", p=P),
            )
            nc.gpsimd.dma_start(
                wout_sb, wout_d.rearrange("(ho p) c -> p ho c", p=P)
            )

            with tc.tile_pool(name="x32pool", bufs=2) as x32pool, tc.tile_pool(
                name="xgpool", bufs=2
            ) as xgpool, tc.tile_pool(name="x8pool", bufs=3) as x8pool:
                # raw fp32 loads, alternating HWDGE queues; "(p tl) c"
                # gives each partition GRP consecutive token rows = 12KB
                # contiguous per descriptor (permutation inverted at y).
                x32s = []
                for g in range(GT):
                    x32 = x32pool.tile([P, GRP, D], F32, tag="x32", name="x32")
                    eng = nc.sync if g % 2 == 0 else nc.scalar
                    eng.dma_start(
                        x32,
                        x_d[g * GRP * P : (g + 1) * GRP * P, :].rearrange(
                            "(p tl) c -> p tl c", p=P
                        ),
                    )
                    x32s.append(x32)

                # ---- phase A: casts + XBAR + C = x^T x (fp8 DR) ----
                with tc.tile_pool(name="psC", bufs=1, space="PSUM") as psC:
                    cps = [
                        psC.tile([P, D - 128 * i], F32, name=f"cps{i}")
                        for i in range(KT)
                    ]
                    for g in range(GT):
                        xg = xgpool.tile([P, GRP, D], FP16, tag="xg", name="xg")
                        if g % 2 == 0:
                            nc.vector.tensor_copy(xg, x32s[g])
                        else:
                            nc.scalar.activation(xg, x32s[g], Copy)
                        if C_FP8:
                            x8 = x8pool.tile([P, GRP, D], FP8, tag="x8", name="x8")
                            if g % 2 == 0:
                                nc.scalar.activation(x8, xg, Copy)
                            else:
                                nc.vector.tensor_copy(x8, xg)
                        xq = nc.sync if g % 2 == 0 else nc.scalar
                        xq.dma_start_transpose(
                            xtt[:, g * GRP : (g + 1) * GRP, :, :].rearrange(
                                "p tl k n -> p (tl k) n"
                            ),
                            xg.rearrange("p tl c -> p (tl c)"),
                        )
                        # wqk fp8 casts mid-stream (wqk lands ~15us)
                        if g == 4:
                            for s in range(KT):
                                if s % 2 == 0:
                                    nc.vector.tensor_copy(
                                        wqk8[:, s, :], wqk_sb[:, s, :]
                                    )
                                else:
                                    nc.scalar.activation(
                                        wqk8[:, s, :], wqk_sb[:, s, :], Copy
                                    )
                        if C_FP8:
                            for j2 in range(GRP // 2):
                                s = 2 * g + j2
                                lpair = x8[:, 2 * j2 : 2 * j2 + 2, :]
                                for i in range(KT):
                                    w = D - 128 * i
                                    for lo in range(0, w, 512):
                                        hi = min(lo + 512, w)
                                        nc.tensor.matmul(
                                            cps[i][:, lo:hi],
                                            lpair[:, :, i * P : (i + 1) * P],
                                            lpair[:, :, 128 * i + lo : 128 * i + hi],
                                            start=(s == 0),
                                            stop=(s == 2 * GT - 1),
                                            perf_mode=DR,
                                        )
                        else:
                            for j in range(GRP):
                                t = GRP * g + j
                                xb = xg[:, j, :]
                                for i in range(KT):
                                    w = D - 128 * i
                                    for lo in range(0, w, 512):
                                        hi = min(lo + 512, w)
                                        nc.tensor.matmul(
                                            cps[i][:, lo:hi],
                                            xb[:, i * P : (i + 1) * P],
                                            xb[:, 128 * i + lo : 128 * i + hi],
                                            start=(t == 0),
                                            stop=(t == TT - 1),
                                        )
                    if USE_WARMACT:
                        nc.scalar.activation(
                            dwarm, dwarm, mybir.ActivationFunctionType.Sqrt, bias=eps2
                        )
                        nc.scalar.activation(
                            dwarm, dwarm, mybir.ActivationFunctionType.Exp
                        )
                    # C PSUM -> fp16 upper (DVE) + fp8/32 upper (ACT)
                    for i in range(KT):
                        nc.vector.tensor_copy(c_sb[:, i, 128 * i : D], cps[i])
                        if USE_ACTC8:
                            nc.scalar.activation(
                                c8_sb[:, i, 128 * i : D], cps[i], Copy, scale=CS
                            )
                        else:
                            nc.vector.tensor_scalar_mul(
                                c8_sb[:, i, 128 * i : D], cps[i], CS
                            )

                # mirror the lower block-triangle: (j,i) = (i,j)^T
                with tc.tile_pool(name="psTP", bufs=3, space="PSUM") as psTP:
                    for i in range(KT):
                        for j in range(i + 1, KT):
                            tpm = psTP.tile([P, P], FP16, tag="tp", name="tpm")
                            nc.tensor.transpose(
                                tpm, c_sb[:, i, j * P : (j + 1) * P], ident16
                            )
                            nc.vector.tensor_scalar_mul(
                                c8_sb[:, j, i * P : (i + 1) * P], tpm, CS
                            )
            # x staging closed: ~46KB/partition freed

            with tc.tile_pool(name="midpool", bufs=1) as mid, tc.tile_pool(
                name="tmppool", bufs=2
            ) as tmppool:
                dscr = mid.tile([P, HP, P], F32)
                n2c = mid.tile([P, 2 * HP], F32)  # [nk^2 | nq^2] compact
                rc = mid.tile([P, 2 * HP], F32)
                skd = mid.tile([P, HP], F32)
                se_c = mid.tile([P, HP], F32)
                rse = mid.tile([P, HP], F32)
                dmat = mid.tile([P, HP, P], FP16)
                rq_sb = mid.tile([P, HP, P], FP16)
                u1 = mid.tile([P, HP, P], F32)
                e16 = mid.tile([P, HP, P], FP16)
                e16m = mid.tile([P, HP, P], FP16)
                a_all = mid.tile([P, HP, P], FP16)
                at_sb = mid.tile([P, HP, P], FP16)

                # Wv^T via XBAR.  The XBAR read of a gpsimd-DMA-written
                # tile raced the DMA on HW (one fi slice landed stale), so
                # stage through a DVE copy for a solid engine dependency.
                wv2 = mid.tile([P, KT, D], FP16)
                nc.vector.tensor_copy(wv2, wv_sb)
                for fi in range(KT):
                    nc.scalar.dma_start_transpose(
                        wvt_sb[:, :, fi * P : (fi + 1) * P], wv2[:, fi, :]
                    )

                # ---- Mqk = (C/32) @ [Wq|Wk], fp8 DoubleRow ----
                # nk norm reduction interleaved; Mk consumed from PSUM.
                with tc.tile_pool(
                    name="psMQ", bufs=2, space="PSUM"
                ) as psMQ, tc.tile_pool(name="psNK", bufs=1, space="PSUM") as psNK:
                    nrmk = psNK.tile([P, D], F32)
                    for f in range(KT):
                        pa = [
                            psMQ.tile([P, 512], F32, tag=f"pmq{i}", name=f"pmq{i}")
                            for i in range(3)
                        ]
                        for kp in range(3):
                            lhs = c8_sb[:, 2 * kp : 2 * kp + 2, f * P : (f + 1) * P]
                            for ch in range(3):
                                nc.tensor.matmul(
                                    pa[ch],
                                    lhs,
                                    wqk8[:, 2 * kp : 2 * kp + 2, ch * 512 : (ch + 1) * 512],
                                    start=(kp == 0),
                                    stop=(kp == 2),
                                    perf_mode=DR,
                                )
                        nc.vector.tensor_copy(mq_sb[:, f, 0:512], pa[0])
                        nc.vector.tensor_copy(mq_sb[:, f, 512:768], pa[1][:, 0:256])
                        wtk = tmppool.tile([P, D], FP16, tag="wtk", name="wtk")
                        nc.vector.tensor_mul(
                            wtk[:, 0:256], wqk_sb[:, f, D : D + 256], pa[1][:, 256:512]
                        )
                        nc.vector.tensor_mul(
                            wtk[:, 256:768], wqk_sb[:, f, D + 256 : 2 * D], pa[2]
                        )
                        for lo, hi in ((0, 512), (512, 768)):
                            nc.tensor.matmul(
                                nrmk[:, lo:hi],
                                ones16,
                                wtk[:, lo:hi],
                                start=(f == 0),
                                stop=(f == KT - 1),
                            )
                    # nk^2 diagonal -> compact [P, HP] (fused mul+reduce)
                    for hp in range(HP):
                        _diag_extract(
                            nc, dscr[:, hp, :], nrmk[:, hp * P : (hp + 1) * P],
                            ident32, n2c[:, hp : hp + 1],
                        )

                # ---- nq norms + logits G + q-scale broadcast ----
                with tc.tile_pool(name="psG", bufs=1, space="PSUM") as psG:
                    nrmq = psG.tile([P, D], F32)
                    g_ps = psG.tile([P, HP, P], F32)
                    rq_ps = psG.tile([P, D], F32)
                    for f in range(KT):
                        wtq = tmppool.tile([P, D], FP16, tag="wtq", name="wtq")
                        nc.vector.tensor_mul(wtq, wqk_sb[:, f, 0:D], mq_sb[:, f, :])
                        for lo, hi in ((0, 512), (512, 768)):
                            nc.tensor.matmul(
                                nrmq[:, lo:hi],
                                ones16,
                                wtq[:, lo:hi],
                                start=(f == 0),
                                stop=(f == KT - 1),
                            )
                    # G_hp = sum_f Wk[f,hp]^T Mq[f,hp]
                    for hp in range(HP):
                        for f in range(KT):
                            nc.tensor.matmul(
                                g_ps[:, hp, :],
                                wqk_sb[:, f, D + hp * P : D + (hp + 1) * P],
                                mq_sb[:, f, hp * P : (hp + 1) * P],
                                start=(f == 0),
                                stop=(f == KT - 1),
                            )
                    # nq^2 diagonal -> compact; then sqrt+recip on [P,12]
                    for hp in range(HP):
                        _diag_extract(
                            nc, dscr[:, hp, :], nrmq[:, hp * P : (hp + 1) * P],
                            ident32, n2c[:, HP + hp : HP + hp + 1],
                        )
                    nc.scalar.activation(
                        n2c, n2c, mybir.ActivationFunctionType.Sqrt, bias=eps2
                    )
                    nc.vector.reciprocal(rc, n2c)
                    nc.vector.tensor_mul(skd, rc[:, 0:HP], tdiag)
                    # replicate rq over partitions: ones^T @ diag(rq)
                    for hp in range(HP):
                        nc.vector.tensor_scalar_mul(
                            dmat[:, hp, :], ident32, rc[:, HP + hp : HP + hp + 1]
                        )
                    dm_flat = dmat.rearrange("p a b -> p (a b)")
                    for lo, hi in ((0, 512), (512, 768)):
                        nc.tensor.matmul(
                            rq_ps[:, lo:hi],
                            ones16,
                            dm_flat[:, lo:hi],
                            start=True,
                            stop=True,
                        )
                    nc.vector.tensor_copy(rq_sb.rearrange("p a b -> p (a b)"), rq_ps)
                    # u = G * (1/nq)[cols]
                    nc.vector.tensor_mul(u1, g_ps, rq_sb)

                # ---- batched softmax over all 6 head pairs ----
                for hp in range(HP):
                    nc.vector.tensor_scalar_mul(
                        u1[:, hp, :], u1[:, hp, :], skd[:, hp : hp + 1]
                    )
                # |logit| <= temp so exp cannot overflow; no max-subtract
                nc.scalar.activation(e16, u1, mybir.ActivationFunctionType.Exp)
                # fused block-diag mask + row-sum
                for hp in range(HP):
                    if USE_TTR:
                        nc.vector.tensor_tensor_reduce(
                            e16m[:, hp, :],
                            e16[:, hp, :],
                            blockmask,
                            1.0,
                            0.0,
                            op0=mybir.AluOpType.mult,
                            op1=mybir.AluOpType.add,
                            accum_out=se_c[:, hp : hp + 1],
                        )
                    else:
                        nc.vector.tensor_mul(e16m[:, hp, :], e16[:, hp, :], blockmask)
                        nc.vector.tensor_reduce(
                            se_c[:, hp : hp + 1], e16m[:, hp, :],
                            axis=mybir.AxisListType.X, op=mybir.AluOpType.add,
                        )
                nc.vector.reciprocal(rse, se_c)
                for hp in range(HP):
                    nc.vector.tensor_scalar_mul(
                        a_all[:, hp, :], e16m[:, hp, :], rse[:, hp : hp + 1]
                    )

                # ---- A^T + W2 = blockdiag(A) @ Wout ----
                with tc.tile_pool(name="psW2", bufs=2, space="PSUM") as psW2:
                    atp = psW2.tile([P, HP, P], FP16, name="atp")
                    for hp in range(HP):
                        nc.tensor.transpose(atp[:, hp, :], a_all[:, hp, :], ident16)
                    nc.vector.tensor_copy(
                        at_sb.rearrange("p a b -> p (a b)"),
                        atp.rearrange("p a b -> p (a b)"),
                    )
                    for hp in range(HP):
                        w2ps = psW2.tile([P, D], F32, tag="w2ps", name="w2ps")
                        for lo, hi in ((0, 512), (512, 768)):
                            nc.tensor.matmul(
                                w2ps[:, lo:hi],
                                at_sb[:, hp, :],
                                wout_sb[:, hp, lo:hi],
                                start=True,
                                stop=True,
                            )
                        nc.vector.tensor_copy(w2_sb[:, hp, :], w2ps)

                # ---------------- W3 = Wv @ W2 ----------------
                with tc.tile_pool(name="psW3", bufs=2, space="PSUM") as psW3:
                    for fi in range(KT):
                        w3ps = psW3.tile([P, D], F32, tag="w3ps", name="w3ps")
                        for g in range(KT):
                            lhs = wvt_sb[:, g, fi * P : (fi + 1) * P]
                            for lo, hi in ((0, 512), (512, 768)):
                                nc.tensor.matmul(
                                    w3ps[:, lo:hi],
                                    lhs,
                                    w2_sb[:, g, lo:hi],
                                    start=(g == 0),
                                    stop=(g == KT - 1),
                                )
                        nc.vector.tensor_copy(w3_sb[:, fi, :], w3ps)

                if dbg:
                    stg = mid.tile([P, KT, D], F32)
                    nc.vector.tensor_copy(stg, c8_sb)
                    nc.vector.tensor_scalar_mul(stg, stg, 32.0)
                    nc.gpsimd.dma_start(
                        dbg["c8"].rearrange("(ko p) c -> p ko c", p=P), stg
                    )
                    nc.gpsimd.dma_start(
                        dbg["mq"].rearrange("(ko p) c -> p ko c", p=P), mq_sb
                    )
                    nc.gpsimd.dma_start(
                        dbg["w3"].rearrange("(ko p) c -> p ko c", p=P), w3_sb
                    )
                    nc.gpsimd.dma_start(
                        dbg["a"][:, :], a_all.rearrange("p a b -> p (a b)")
                    )
                    nc.gpsimd.dma_start(dbg["nrm"][:, 0 : 2 * HP], n2c[:, :])
                    nc.gpsimd.dma_start(dbg["nrm"][:, 2 * HP : 3 * HP], skd)
                    nc.gpsimd.dma_start(dbg["nrm"][:, 3 * HP : 4 * HP], rse)
                    nc.gpsimd.dma_start(
                        dbg["at"][:, :], at_sb.rearrange("p a b -> p (a b)")
                    )
                    nc.gpsimd.dma_start(
                        dbg["w2"].rearrange("(ko p) c -> p ko c", p=P), w2_sb
                    )
                    nc.gpsimd.dma_start(
                        dbg["wvt"].rearrange("(ko p) c -> p ko c", p=P), wvt_sb
                    )

        # ---------------- phase E: y = x @ W3 + bout --------------------
        with tc.tile_pool(name="ypool", bufs=3) as ypool, tc.tile_pool(
            name="psY", bufs=3, space="PSUM"
        ) as psY:
            for t in range(TT):
                g, tl = divmod(t, GRP)
                yps = psY.tile([P, D], F32, tag="yps", name="yps")
                for k in range(KT):
                    lhs = xtt[:, t, k, :]
                    for lo, hi in ((0, 512), (512, 768)):
                        nc.tensor.matmul(
                            yps[:, lo:hi],
                            lhs,
                            w3_sb[:, k, lo:hi],
                            start=(k == 0),
                            stop=(k == KT - 1),
                        )
                ysb = ypool.tile([P, D], F32, tag="ysb", name="ysb")
                nc.vector.tensor_add(ysb, yps, bout_sb)
                # invert the load permutation: partition p holds token
                # g*512 + 4p + tl
                yv = y_d[g * GRP * P : (g + 1) * GRP * P, :].rearrange(
                    "(p tl) c -> p tl c", p=P
                )[:, tl, :]
                nc.sync.dma_start(yv, ysb)


_NC_CACHE = {}


def _get_nc():
    if "nc" not in _NC_CACHE:
        _NC_CACHE["nc"] = build_nc()
    return _NC_CACHE["nc"]


def kernel_with_results(x, Wqkv, temperature, Wout, bout, **run_kwargs):
    x = np.ascontiguousarray(np.asarray(x, dtype=np.float32))
    Wqkv = np.ascontiguousarray(np.asarray(Wqkv, dtype=np.float32))
    temp = np.ascontiguousarray(np.asarray(temperature, dtype=np.float32).reshape(H))
    Wout = np.ascontiguousarray(np.asarray(Wout, dtype=np.float32))
    bout = np.ascontiguousarray(np.asarray(bout, dtype=np.float32))

    nc = _get_nc()
    in_maps = [
        {"x": x[b], "wqkv": Wqkv, "temp": temp, "wout": Wout, "bout": bout}
        for b in range(B)
    ]
    res = run_bass_kernel_spmd(nc, in_maps, core_ids=list(range(B)), **run_kwargs)
    out = np.stack([r["y"] for r in res.results], axis=0)
    return out, res


def kernel(x, Wqkv, temperature, Wout, bout):
    out, _ = kernel_with_results(x, Wqkv, temperature, Wout, bout)
    return out
